# revision 1
# baseline (speedup 1.0000x reference)
"""Trainium2 Bass kernel for (W0 (x) W1 (x) W2 (x) W3) @ x  -- Kronecker chain.

Shapes: x [2^20, 32] fp32, Wi [32, 32] fp32. Output [2^20, 32] fp32.
Batch-sharded across 8 cores (core c owns x[:, 4c:4c+4]).

Same PE-crossing architecture as the original baseline (stages 1-3 run the
matmul data-stationary with augmented delta-structured weights as the moving
operand so each contraction rotates the next mode onto the partition axis;
stage 4 is weight-stationary), with a rescheduled back end:

  * PSUM->SBUF evacuations are the global bottleneck: GPSIMD and DMA have no
    PSUM port on trn2, so ~131k elem/partition must flow through Act+DVE at
    1 elem/cycle each.  Copies are balanced across the two engines with a
    running-deficit balancer (Act is ~15% faster per copy).
  * Software-pipelined emission: S2 lags S1 by 2 chunks and S4 lags S3 by
    2 slots, so no matmul ever waits on the evacuation of its own chunk and
    both copy engines stay saturated.
  * Loads run 3 chunks ahead; stores fire per 1024-column block.
  * All pools stay open across both phases (flat ExitStack scope); all four
    stages share one 4-buffer PSUM ring and t1 is deeply buffered (12) --
    measured faster than phase-scoped 2+2 psum pools (more rotation slack,
    no pool churn at the phase barrier).
"""
import numpy as np
import ml_dtypes

import concourse.bass as bass
import concourse.bacc as bacc
import concourse.mybir as mybir
import contextlib
import concourse.tile as tile
from concourse.bass_utils import run_bass_kernel_spmd

F32 = mybir.dt.float32
BF16 = mybir.dt.bfloat16

L = 32
N = L ** 4          # 1048576
B = 32
NCORES = 8
BC = B // NCORES    # 4

S2_LAG = 2
S4_LAG = 2
LOAD_AHEAD = 3
ACT_NS = 1038.0
DVE_NS = 1192.0

_NC_CACHE = {}


def _build_nc():
    nc = bacc.Bacc("TRN2", target_bir_lowering=False, debug=False)

    # x pre-shuffled on host to [j0, (j1h, j2):128, (j1l, j3, b):1024], bf16
    x = nc.dram_tensor("x", [32, 131072], BF16, kind="ExternalInput")
    w2a = nc.dram_tensor("w2a", [128, 128], BF16, kind="ExternalInput")
    w3a = nc.dram_tensor("w3a", [128, 128], BF16, kind="ExternalInput")
    w1a = nc.dram_tensor("w1a", [128, 128], BF16, kind="ExternalInput")
    w0a = nc.dram_tensor("w0a", [128, 128], BF16, kind="ExternalInput")
    # y device order: [i2a(8), (i0, i3b)(128), (i3a, b, i1, i2b)(4096)] bf16
    y = nc.dram_tensor("y", [8, 524288], BF16, kind="ExternalOutput")

    busy = {"A": 900.0, "D": 0.0}

    def evac(out_ap, in_ap):
        # running-deficit balance across the two PSUM-capable copy engines
        # (init offsets let DVE take the first copy of each phase: measured
        # faster in TimelineSim than Act-first)
        if busy["A"] + ACT_NS <= busy["D"] + DVE_NS:
            busy["A"] += ACT_NS
            nc.scalar.copy(out=out_ap, in_=in_ap)
        else:
            busy["D"] += DVE_NS
            nc.vector.tensor_copy(out_ap, in_ap)

    with tile.TileContext(nc) as tc:
        with tc.tile_pool(name="wp", bufs=1) as wp, \
             tc.tile_pool(name="b1p", bufs=1) as b1p:
            w2s = wp.tile([128, 128], BF16, name="w2s")
            w3s = wp.tile([128, 128], BF16, name="w3s")
            w1s = wp.tile([128, 128], BF16, name="w1s")
            w0s = wp.tile([128, 128], BF16, name="w0s")

            # B1: [part (i2b,j1), addr = i2a*4096 + i3a*512 + b*128
            #      + i3b*32 + j0] bf16
            b1 = b1p.tile([128, 32768], BF16, name="b1")
            b1_t, b1_o = b1.tensor, b1.offset

            # ---- Phase I: S1 (contract j2) + S2 (contract j3), per j0 ----
            _st = contextlib.ExitStack()
            lp = _st.enter_context(tc.tile_pool(name="lp", bufs=4))
            t1p = _st.enter_context(tc.tile_pool(name="t1p", bufs=12))
            t3p = _st.enter_context(tc.tile_pool(name="t3p", bufs=3))
            stgp = _st.enter_context(tc.tile_pool(name="stgp", bufs=5))
            ps1 = _st.enter_context(tc.tile_pool(name="ps1", bufs=4, space="PSUM"))
            ps2 = ps1
            if True:
                lts = {}
                t1s = {}

                def load(c):
                    lt = lp.tile([128, 1024], BF16, name="lt")
                    nc.sync.dma_start(
                        out=lt[:],
                        in_=bass.AP(x, c * 131072, [[1024, 128], [1, 1024]]))
                    lts[c] = lt

                def s1(c):
                    lt = lts.pop(c)
                    lt_t, lt_o = lt.tensor, lt.offset
                    p1 = ps1.tile([128, 1024], F32, name="p1", tag="a")
                    for j1l in range(8):
                        lhsT = bass.AP(lt_t, lt_o + j1l * 128,
                                       [[1024, 128], [1, 128]])
                        nc.tensor.matmul(p1[:, j1l * 128:(j1l + 1) * 128],
                                         lhsT, w2s[:], start=True, stop=True)
                    # psum pos (j1l, n1=(i2a,i2b,j1h)); merge (i2b,j1h)->[8,16]
                    t1 = t1p.tile([128, 1024], BF16, name="t1")
                    t1_ap = bass.AP(t1.tensor, t1.offset,
                                    [[1024, 128], [1, 8], [128, 8], [8, 16]])
                    if c == 0:
                        # Act takes the very first copy (against the deficit
                        # rule): overlaps its weight-DMA dispatch window and
                        # shifts every later assignment one notch -- measured
                        # -367 ns in TimelineSim
                        busy["A"] += ACT_NS
                        nc.scalar.copy(out=t1_ap, in_=p1[:])
                    else:
                        evac(t1_ap, p1[:])
                    t1s[c] = t1

                def s2(c):
                    t1 = t1s.pop(c)
                    t1_t, t1_o = t1.tensor, t1.offset
                    p2 = ps2.tile([128, 1024], F32, name="p2", tag="a")
                    for i2a in range(8):
                        lhsT = bass.AP(t1_t, t1_o + i2a * 128,
                                       [[1024, 128], [1, 128]])
                        nc.tensor.matmul(p2[:, i2a * 128:(i2a + 1) * 128],
                                         lhsT, w3s[:], start=True, stop=True)
                    # psum pos (i2a, n2=(i3a,b,i3b)); merge (b,i3b)->[32,16]
                    if c == 31:
                        # split the barrier-critical last chunk across both
                        # engines: halves the latency phase II waits on
                        nc.scalar.copy(
                            out=bass.AP(b1_t, b1_o + c,
                                        [[32768, 128], [4096, 4], [512, 8], [32, 16]]),
                            in_=p2[:, :512])
                        nc.vector.tensor_copy(
                            bass.AP(b1_t, b1_o + c + 16384,
                                    [[32768, 128], [4096, 4], [512, 8], [32, 16]]),
                            p2[:, 512:])
                    else:
                        evac(bass.AP(b1_t, b1_o + c,
                                     [[32768, 128], [4096, 8], [512, 8], [32, 16]]),
                             p2[:])

                # weight loads issue from Act's HWDGE queue so their
                # dispatch overlaps the SP-issued x loads
                load(0)
                nc.scalar.dma_start(out=w2s[:], in_=w2a.ap())
                nc.scalar.dma_start(out=w3s[:], in_=w3a.ap())
                load(1)
                load(2)
                nc.scalar.dma_start(out=w1s[:], in_=w1a.ap())
                nc.scalar.dma_start(out=w0s[:], in_=w0a.ap())
                for c in range(32 + S2_LAG):
                    if c < 32:
                        if c + LOAD_AHEAD < 32:
                            load(c + LOAD_AHEAD)
                        s1(c)
                    if c >= S2_LAG:
                        s2(c - S2_LAG)

            # ---- Phase II: S3 + S4 (same pools; shared tags keep psum at 8 banks)
            if True:
                ps3, ps4 = ps1, ps2
                t3s = {}
                busy["A"], busy["D"] = 800.0, 0.0

                def s3(k, th):
                    # T3: [part (i3b,j0), free (i3a:512, b:128, (i1*4+i2b):1)]
                    if th == 0:
                        t3s[k] = t3p.tile([128, 4096], BF16, name="t3")
                    t3 = t3s[k]
                    p3 = ps3.tile([128, 1024], F32, name="p3", tag="a")
                    for q in range(8):
                        cq = 8 * th + q      # cq = i3a*4 + b
                        lhsT = bass.AP(b1_t, b1_o + k * 4096 + cq * 128,
                                       [[32768, 128], [1, 128]])
                        nc.tensor.matmul(p3[:, q * 128:(q + 1) * 128],
                                         lhsT, w1s[:], start=True, stop=True)
                    evac(t3[:, th * 1024:(th + 1) * 1024], p3[:])

                def s4(k, th):
                    # reads t3 block [th*1024, (th+1)*1024) = i3a in {2th,2th+1}
                    t3 = t3s[k]
                    t3_t, t3_o = t3.tensor, t3.offset
                    p4 = ps4.tile([128, 1024], F32, name="p4", tag="a")
                    for m in range(2):
                        i3a = 2 * th + m
                        rhs = bass.AP(t3_t, t3_o + i3a * 512,
                                      [[4096, 128], [128, 4], [1, 128]])
                        nc.tensor.matmul(p4[:, m * 512:(m + 1) * 512],
                                         w0s[:], rhs, start=True, stop=True)
                    if th == 3:
                        t3s.pop(k)
                    stg = stgp.tile([128, 1024], BF16, name="stg")
                    evac(stg[:], p4[:])
                    nc.sync.dma_start(
                        out=bass.AP(y, k * 524288 + th * 1024,
                                    [[4096, 128], [1, 1024]]),
                        in_=stg[:])

                slots = [(k, th) for k in range(8) for th in range(4)]
                for i in range(32 + S4_LAG):
                    if i < 32:
                        s3(*slots[i])
                    if i >= S4_LAG:
                        s4(*slots[i - S4_LAG])

            _st.close()

    nc.finalize()
    return nc


def _build_waug(w: np.ndarray, kind: str) -> np.ndarray:
    """Augmented 128x128 weights (4-way replicated, delta-structured)."""
    wa = np.zeros((128, 128), dtype=np.float32)
    ar = np.arange(32)
    if kind == "w3":
        # rows p = j3*4 + b ; cols n = i3a*16 + b*4 + i3b
        for b in range(4):
            cols = (ar >> 2) * 16 + b * 4 + (ar & 3)
            wa[np.ix_(ar * 4 + b, cols)] = w.T
    else:
        # rows p = q*32 + j ; cols n = i*4 + q
        for q in range(4):
            wa[np.ix_(q * 32 + ar, ar * 4 + q)] = w.T
    return wa


def _get_nc():
    if "nc" not in _NC_CACHE:
        _NC_CACHE["nc"] = _build_nc()
    return _NC_CACHE["nc"]


def make_in_maps(x, W0, W1, W2, W3):
    x = np.asarray(x, dtype=np.float32)
    bf = ml_dtypes.bfloat16
    w2a = _build_waug(np.asarray(W2, np.float32), "q").astype(bf)
    w3a = _build_waug(np.asarray(W3, np.float32), "w3").astype(bf)
    w1a = _build_waug(np.asarray(W1, np.float32), "q").astype(bf)
    w0a = _build_waug(np.asarray(W0, np.float32), "q").astype(bf)
    xr = x.reshape(32, 4, 8, 32, 32, B)
    in_maps = []
    for c in range(NCORES):
        xc = xr[..., c * BC:(c + 1) * BC].transpose(0, 1, 3, 2, 4, 5)
        xc = np.ascontiguousarray(xc).astype(bf).reshape(32, 131072)
        in_maps.append({"x": xc, "w2a": w2a, "w3a": w3a,
                        "w1a": w1a, "w0a": w0a})
    return in_maps


def _unshuffle_y(yd: np.ndarray) -> np.ndarray:
    """[i2a(8), (i0, i3b), (i3a, b, i1, i2b)] -> [N, BC]."""
    y = yd.astype(np.float32).reshape(8, 32, 4, 8, BC, 32, 4)
    y = y.transpose(1, 5, 0, 6, 3, 2, 4)
    return np.ascontiguousarray(y).reshape(N, BC)


def kernel(x, W0, W1, W2, W3, _trace=False):
    nc = _get_nc()
    in_maps = make_in_maps(x, W0, W1, W2, W3)
    res = run_bass_kernel_spmd(nc, in_maps, core_ids=list(range(NCORES)),
                               trace=_trace)
    out = np.concatenate(
        [_unshuffle_y(res.results[c]["y"]) for c in range(NCORES)], axis=1)
    if _trace:
        kernel.last_result = res
    return out


if __name__ == "__main__":
    rng = np.random.default_rng(0)
    x = rng.standard_normal((N, B), dtype=np.float32)
    ws = [rng.standard_normal((L, L), dtype=np.float32) for _ in range(4)]
    y = kernel(x, *ws)
    print("ran", y.shape, y.dtype)



# revision 34
# speedup vs baseline: 1.0081x; 1.0081x over previous
"""Trainium2 Bass kernel for (W0 (x) W1 (x) W2 (x) W3) @ x  -- Kronecker chain.

Shapes: x [2^20, 32] fp32, Wi [32, 32] fp32. Output [2^20, 32] fp32.
Batch-sharded across 8 cores (core c owns x[:, 4c:4c+4]).

Same PE-crossing architecture as the original baseline (stages 1-3 run the
matmul data-stationary with augmented delta-structured weights as the moving
operand so each contraction rotates the next mode onto the partition axis;
stage 4 is weight-stationary), with a rescheduled back end:

  * PSUM->SBUF evacuations are the global bottleneck: GPSIMD and DMA have no
    PSUM port on trn2, so ~131k elem/partition must flow through Act+DVE at
    1 elem/cycle each.  Copies are balanced across the two engines with a
    running-deficit balancer (Act is ~15% faster per copy).
  * Software-pipelined emission: S2 lags S1 by 2 chunks and S4 lags S3 by
    2 slots, so no matmul ever waits on the evacuation of its own chunk and
    both copy engines stay saturated.
  * Loads run 3 chunks ahead; stores fire per 1024-column block.
  * All pools stay open across both phases (flat ExitStack scope); all four
    stages share one 4-buffer PSUM ring and t1 is deeply buffered (12) --
    measured faster than phase-scoped 2+2 psum pools (more rotation slack,
    no pool churn at the phase barrier).
"""
import numpy as np
import ml_dtypes

import concourse.bass as bass
import concourse.bacc as bacc
import concourse.mybir as mybir
import contextlib
import concourse.tile as tile
from concourse.bass_utils import run_bass_kernel_spmd

F32 = mybir.dt.float32
BF16 = mybir.dt.bfloat16

L = 32
N = L ** 4          # 1048576
B = 32
NCORES = 8
BC = B // NCORES    # 4

S2_LAG = 2
S4_LAG = 2
LOAD_AHEAD = 3
ACT_NS = 1038.0
DVE_NS = 1192.0

_NC_CACHE = {}

# tuning toggles (bisected with TimelineSim; see test.py/analyze.py)
OPTS = {
    "head": False,    # c0/c1 per-bank evac halves
    "wdefer": False,  # weight dma dispatch on gpsimd queue (off Act SEQ)
    "s1_31": False,   # split s1(31) evac across engines
    "s3_head": False, # split s3(0,0)/(0,1) evac across engines
    "s3_tail": False, # split s3(7,3) evac across engines
    "s4_tail": False, # split s4(7,>=2) copies+stores
    "lp": 4,          # x-chunk load pool depth
    "la": 3,          # load-ahead distance
    "s2lag": 2,       # s2 emission lag behind s1
    "s4lag": 2,       # s4 emission lag behind s3
    "t1p": 12,        # t1 pool depth
    "stgp": 5,        # store staging pool depth
    "offA": 900.0,    # phase-I balancer offset for Act
    "offA2": 800.0,   # phase-II balancer offset for Act
    "dualload": False,   # alternate x loads across SP and gpsimd queues
    "dualstore": False,  # alternate y stores across SP and gpsimd queues
    "st_split": False,   # split final slots' stores (copies intact)
    "tailA": False,      # force the last s4 copy onto Act (faster engine)
    "lastg": False,      # issue the final slot's store from the idle gpsimd queue
    "t3p": 3,            # t3 pool depth
}


def _build_nc():
    nc = bacc.Bacc("TRN2", target_bir_lowering=False, debug=False)

    # x pre-shuffled on host to [j0, (j1h, j2):128, (j1l, j3, b):1024], bf16
    x = nc.dram_tensor("x", [32, 131072], BF16, kind="ExternalInput")
    w2a = nc.dram_tensor("w2a", [128, 128], BF16, kind="ExternalInput")
    w3a = nc.dram_tensor("w3a", [128, 128], BF16, kind="ExternalInput")
    w1a = nc.dram_tensor("w1a", [128, 128], BF16, kind="ExternalInput")
    w0a = nc.dram_tensor("w0a", [128, 128], BF16, kind="ExternalInput")
    # y device order: [i2a(8), (i0, i3b)(128), (i3a, b, i1, i2b)(4096)] bf16
    y = nc.dram_tensor("y", [8, 524288], BF16, kind="ExternalOutput")

    busy = {"A": OPTS["offA"], "D": 0.0}

    def act_cost(n):
        return (n + 222.0) / 1.2

    def dve_cost(n):
        return (n + 120.0) / 0.96

    def evac(out_ap, in_ap):
        # running-deficit balance across the two PSUM-capable copy engines
        # (init offsets let DVE take the first copy of each phase: measured
        # faster in TimelineSim than Act-first)
        n = in_ap.free_size()
        if busy["A"] + act_cost(n) <= busy["D"] + dve_cost(n):
            busy["A"] += act_cost(n)
            nc.scalar.copy(out=out_ap, in_=in_ap)
        else:
            busy["D"] += dve_cost(n)
            nc.vector.tensor_copy(out_ap, in_ap)

    def copy_on(eng, out_ap, in_ap):
        n = in_ap.free_size()
        if eng == "A":
            busy["A"] += act_cost(n)
            nc.scalar.copy(out=out_ap, in_=in_ap)
        else:
            busy["D"] += dve_cost(n)
            nc.vector.tensor_copy(out_ap, in_ap)

    def evac_split(pieces):
        # latency-critical evacuation: force pieces to alternate engines so
        # both PSUM ports drain the tile concurrently
        first = "A" if busy["A"] <= busy["D"] else "D"
        other = "D" if first == "A" else "A"
        for i, (o, inp) in enumerate(pieces):
            copy_on(first if i % 2 == 0 else other, o, inp)

    with tile.TileContext(nc) as tc:
        with tc.tile_pool(name="wp", bufs=1) as wp, \
             tc.tile_pool(name="b1p", bufs=1) as b1p:
            w2s = wp.tile([128, 128], BF16, name="w2s")
            w3s = wp.tile([128, 128], BF16, name="w3s")
            w1s = wp.tile([128, 128], BF16, name="w1s")
            w0s = wp.tile([128, 128], BF16, name="w0s")

            # B1: [part (i2b,j1), addr = i2a*4096 + i3a*512 + b*128
            #      + i3b*32 + j0] bf16
            b1 = b1p.tile([128, 32768], BF16, name="b1")
            b1_t, b1_o = b1.tensor, b1.offset

            # ---- Phase I: S1 (contract j2) + S2 (contract j3), per j0 ----
            _st = contextlib.ExitStack()
            lp = _st.enter_context(tc.tile_pool(name="lp", bufs=OPTS["lp"]))
            t1p = _st.enter_context(tc.tile_pool(name="t1p", bufs=OPTS["t1p"]))
            t3p = _st.enter_context(tc.tile_pool(name="t3p", bufs=OPTS["t3p"]))
            stgp = _st.enter_context(tc.tile_pool(name="stgp", bufs=OPTS["stgp"]))
            ps1 = _st.enter_context(tc.tile_pool(name="ps1", bufs=4, space="PSUM"))
            ps2 = ps1
            if True:
                lts = {}
                t1s = {}

                def load(c):
                    lt = lp.tile([128, 1024], BF16, name="lt")
                    # alternate issue queues: each dma_start holds its
                    # sequencer ~1.26us through HWDGE, so one queue caps the
                    # load stream at ~1.26us/chunk -- two queues halve that
                    q = nc.gpsimd if (OPTS["dualload"] and c % 2) else nc.sync
                    q.dma_start(
                        out=lt[:],
                        in_=bass.AP(x, c * 131072, [[1024, 128], [1, 1024]]))
                    lts[c] = lt

                def s1(c):
                    lt = lts.pop(c)
                    lt_t, lt_o = lt.tensor, lt.offset
                    p1 = ps1.tile([128, 1024], F32, name="p1", tag="a")
                    t1 = t1p.tile([128, 1024], BF16, name="t1")
                    t1_t, t1_o = t1.tensor, t1.offset

                    def mm(j1l):
                        lhsT = bass.AP(lt_t, lt_o + j1l * 128,
                                       [[1024, 128], [1, 128]])
                        nc.tensor.matmul(p1[:, j1l * 128:(j1l + 1) * 128],
                                         lhsT, w2s[:], start=True, stop=True)

                    def t1_ap(j1l0, nj):
                        # psum (j1l, i2a, (i2b,j1h)) -> t1 (i2, j1h, j1l)
                        return bass.AP(t1_t, t1_o + j1l0,
                                       [[1024, 128], [1, nj], [128, 8], [8, 16]])

                    if c <= 1 and OPTS["head"]:
                        # pipeline head: evacuate per PSUM bank (512 fp32 is
                        # the min split - Tile's tracker is bank-aware, so a
                        # finer split serializes PE against the copy engines)
                        # so the copy engines start ~1.4us earlier
                        for j1l in range(4):
                            mm(j1l)
                        copy_on("A" if c == 0 else "D",
                                t1_ap(0, 4), p1[:, :512])
                        for j1l in range(4, 8):
                            mm(j1l)
                        copy_on("D" if c == 0 else "A",
                                t1_ap(4, 4), p1[:, 512:])
                    else:
                        for j1l in range(8):
                            mm(j1l)
                        if c == 0:
                            busy["A"] += act_cost(1024)
                            nc.scalar.copy(out=t1_ap(0, 8), in_=p1[:])
                        elif c == 31 and OPTS["s1_31"]:
                            # barrier-critical: drain with both engines
                            evac_split([
                                (t1_ap(0, 4), p1[:, :512]),
                                (t1_ap(4, 4), p1[:, 512:]),
                            ])
                        else:
                            evac(t1_ap(0, 8), p1[:])
                    t1s[c] = t1

                def s2(c):
                    t1 = t1s.pop(c)
                    t1_t, t1_o = t1.tensor, t1.offset
                    p2 = ps2.tile([128, 1024], F32, name="p2", tag="a")
                    for i2a in range(8):
                        lhsT = bass.AP(t1_t, t1_o + i2a * 128,
                                       [[1024, 128], [1, 128]])
                        nc.tensor.matmul(p2[:, i2a * 128:(i2a + 1) * 128],
                                         lhsT, w3s[:], start=True, stop=True)
                    # psum pos (i2a, n2=(i3a,b,i3b)); merge (b,i3b)->[32,16]
                    if c == 31:
                        # barrier-critical last chunk: split per PSUM bank
                        # across both engines so phase II unblocks earliest
                        evac_split([
                            (bass.AP(b1_t, b1_o + c + 4 * h * 4096,
                                     [[32768, 128], [4096, 4], [512, 8], [32, 16]]),
                             p2[:, h * 512:(h + 1) * 512])
                            for h in range(2)
                        ])
                    else:
                        evac(bass.AP(b1_t, b1_o + c,
                                     [[32768, 128], [4096, 8], [512, 8], [32, 16]]),
                             p2[:])

                # weight loads issue from the otherwise-idle GPSIMD queue
                # (SWDGE): keeping them off Act's in-order SEQ saves ~1us --
                # the Act-queue HWDGE dispatches blocked the first evacuation
                # behind 4x ~1.25us of SEQ-held DMA setup
                wq = nc.gpsimd if OPTS["wdefer"] else nc.scalar
                load(0)
                wq.dma_start(out=w2s[:], in_=w2a.ap())
                wq.dma_start(out=w3s[:], in_=w3a.ap())
                load(1)
                load(2)
                wq.dma_start(out=w1s[:], in_=w1a.ap())
                wq.dma_start(out=w0s[:], in_=w0a.ap())
                for c in range(3, min(OPTS["la"], 32)):
                    load(c)
                S2L = OPTS["s2lag"]
                for c in range(32 + S2L):
                    if c < 32:
                        if c + OPTS["la"] < 32:
                            load(c + OPTS["la"])
                        s1(c)
                    if c >= S2L:
                        s2(c - S2L)

            # ---- Phase II: S3 + S4 (same pools; shared tags keep psum at 8 banks)
            if True:
                ps3, ps4 = ps1, ps2
                t3s = {}
                busy["A"], busy["D"] = OPTS["offA2"], 0.0

                def s3(k, th):
                    # T3: [part (i3b,j0), free (i3a:512, b:128, (i1*4+i2b):1)]
                    if th == 0:
                        t3s[k] = t3p.tile([128, 4096], BF16, name="t3")
                    t3 = t3s[k]
                    p3 = ps3.tile([128, 1024], F32, name="p3", tag="a")
                    for q in range(8):
                        cq = 8 * th + q      # cq = i3a*4 + b
                        lhsT = bass.AP(b1_t, b1_o + k * 4096 + cq * 128,
                                       [[32768, 128], [1, 128]])
                        nc.tensor.matmul(p3[:, q * 128:(q + 1) * 128],
                                         lhsT, w1s[:], start=True, stop=True)
                    if (k == 0 and th <= 1 and OPTS["s3_head"]) or (
                            k == 7 and th == 3 and OPTS["s3_tail"]):
                        # post-barrier refill / tail-critical: split so both
                        # engines work the latency-critical tile concurrently
                        evac_split([
                            (t3[:, th * 1024:th * 1024 + 512], p3[:, :512]),
                            (t3[:, th * 1024 + 512:(th + 1) * 1024], p3[:, 512:]),
                        ])
                    else:
                        evac(t3[:, th * 1024:(th + 1) * 1024], p3[:])

                def s4(k, th):
                    # reads t3 block [th*1024, (th+1)*1024) = i3a in {2th,2th+1}
                    t3 = t3s[k]
                    t3_t, t3_o = t3.tensor, t3.offset
                    p4 = ps4.tile([128, 1024], F32, name="p4", tag="a")
                    for m in range(2):
                        i3a = 2 * th + m
                        rhs = bass.AP(t3_t, t3_o + i3a * 512,
                                      [[4096, 128], [128, 4], [1, 128]])
                        nc.tensor.matmul(p4[:, m * 512:(m + 1) * 512],
                                         w0s[:], rhs, start=True, stop=True)
                    if th == 3:
                        t3s.pop(k)
                    stg = stgp.tile([128, 1024], BF16, name="stg")
                    slot = 4 * k + th
                    sq = nc.gpsimd if (OPTS["dualstore"] and slot % 2) else nc.sync
                    if OPTS["lastg"] and slot == 31:
                        sq = nc.gpsimd
                    if k == 7 and th >= 2 and OPTS["s4_tail"]:
                        # tail: per-bank copies with per-half stores so the
                        # final store's DMA latency chain starts earlier
                        first = "A" if busy["A"] <= busy["D"] else "D"
                        other = "D" if first == "A" else "A"
                        for h in range(2):
                            copy_on(first if h == 0 else other,
                                    stg[:, h * 512:(h + 1) * 512],
                                    p4[:, h * 512:(h + 1) * 512])
                            sq.dma_start(
                                out=bass.AP(y, k * 524288 + th * 1024 + h * 512,
                                            [[4096, 128], [1, 512]]),
                                in_=stg[:, h * 512:(h + 1) * 512])
                    else:
                        if k == 7 and th == 3 and OPTS["tailA"]:
                            copy_on("A", stg[:], p4[:])
                        else:
                            evac(stg[:], p4[:])
                        if k == 7 and th >= 2 and OPTS["st_split"]:
                            # single copy, two stores: the first half's store
                            # chain overlaps; the last transfer halves
                            for h in range(2):
                                sq.dma_start(
                                    out=bass.AP(y,
                                                k * 524288 + th * 1024 + h * 512,
                                                [[4096, 128], [1, 512]]),
                                    in_=stg[:, h * 512:(h + 1) * 512])
                        else:
                            sq.dma_start(
                                out=bass.AP(y, k * 524288 + th * 1024,
                                            [[4096, 128], [1, 1024]]),
                                in_=stg[:])

                slots = [(k, th) for k in range(8) for th in range(4)]
                S4L = OPTS["s4lag"]
                for i in range(32 + S4L):
                    if i < 32:
                        s3(*slots[i])
                    if i >= S4L:
                        s4(*slots[i - S4L])

            _st.close()

    nc.finalize()
    return nc


def _build_waug(w: np.ndarray, kind: str) -> np.ndarray:
    """Augmented 128x128 weights (4-way replicated, delta-structured)."""
    wa = np.zeros((128, 128), dtype=np.float32)
    ar = np.arange(32)
    if kind == "w3":
        # rows p = j3*4 + b ; cols n = i3a*16 + b*4 + i3b
        for b in range(4):
            cols = (ar >> 2) * 16 + b * 4 + (ar & 3)
            wa[np.ix_(ar * 4 + b, cols)] = w.T
    else:
        # rows p = q*32 + j ; cols n = i*4 + q
        for q in range(4):
            wa[np.ix_(q * 32 + ar, ar * 4 + q)] = w.T
    return wa


def _get_nc():
    if "nc" not in _NC_CACHE:
        _NC_CACHE["nc"] = _build_nc()
    return _NC_CACHE["nc"]


def make_in_maps(x, W0, W1, W2, W3):
    x = np.asarray(x, dtype=np.float32)
    bf = ml_dtypes.bfloat16
    w2a = _build_waug(np.asarray(W2, np.float32), "q").astype(bf)
    w3a = _build_waug(np.asarray(W3, np.float32), "w3").astype(bf)
    w1a = _build_waug(np.asarray(W1, np.float32), "q").astype(bf)
    w0a = _build_waug(np.asarray(W0, np.float32), "q").astype(bf)
    xr = x.reshape(32, 4, 8, 32, 32, B)
    in_maps = []
    for c in range(NCORES):
        xc = xr[..., c * BC:(c + 1) * BC].transpose(0, 1, 3, 2, 4, 5)
        xc = np.ascontiguousarray(xc).astype(bf).reshape(32, 131072)
        in_maps.append({"x": xc, "w2a": w2a, "w3a": w3a,
                        "w1a": w1a, "w0a": w0a})
    return in_maps


def _unshuffle_y(yd: np.ndarray) -> np.ndarray:
    """[i2a(8), (i0, i3b), (i3a, b, i1, i2b)] -> [N, BC]."""
    y = yd.astype(np.float32).reshape(8, 32, 4, 8, BC, 32, 4)
    y = y.transpose(1, 5, 0, 6, 3, 2, 4)
    return np.ascontiguousarray(y).reshape(N, BC)


def kernel(x, W0, W1, W2, W3, _trace=False):
    nc = _get_nc()
    in_maps = make_in_maps(x, W0, W1, W2, W3)
    res = run_bass_kernel_spmd(nc, in_maps, core_ids=list(range(NCORES)),
                               trace=_trace)
    out = np.concatenate(
        [_unshuffle_y(res.results[c]["y"]) for c in range(NCORES)], axis=1)
    if _trace:
        kernel.last_result = res
    return out


if __name__ == "__main__":
    rng = np.random.default_rng(0)
    x = rng.standard_normal((N, B), dtype=np.float32)
    ws = [rng.standard_normal((L, L), dtype=np.float32) for _ in range(4)]
    y = kernel(x, *ws)
    print("ran", y.shape, y.dtype)

